# revision 16
# baseline (speedup 1.0000x reference)
"""Trainium2 Bass kernel: VAE-style AttnBlock.

  y = x + proj( attention( q(gn(x)), k(gn(x)), v(gn(x)) ) )

  x: [2, 512, 64, 64] f32, gn = GroupNorm(8 groups, eps=1e-6),
  q/k/v/proj = 1x1 convs (512x512), attention over the 4096 spatial
  positions with softmax along the key axis, scale = 512**-0.5.

Sharding: 8 cores = (batch b, query-block qb); each core computes the
softmax rows for its 1024 query positions of batch b against the full
K/V of that batch (K/V conv is recomputed per core - cheaper than a
cross-core exchange at this size). Conv weights replicated.

Folding (host side, exact f32/f64): GroupNorm stats (mean/var per
group per batch) fold into the conv weights; Wq^T@Wk pre-multiplies
into one bilinear matrix A so the S matmul needs a conv on the query
side only; Wp@Wv pre-multiplies so AV directly yields the projected
output.  The k-side bias and v-bias fold into per-query constants /
the output bias (softmax over keys is invariant to per-query shifts).

Device work is exactly the O(n C^2) convs and O(n^2 C) attention:
  VT = x8^T wv8            (proj-fused V, fp8)
  q8 = s/16 (wa8^T x8 + bqe)                 (fused Q, fp8)
  S^T = x8^T q8 ; at = exp(S/sqrt(C) - 3)    (fp8)
  cs  = ones^T at          (softmax normalizer, PE-accumulated)
  O   = (VT^T at) / cs + bpe + x             (f32 epilogue)

All large matmuls run fp8 (e4m3) with DoubleRow perf mode - the PE
packs two fp8 weights per cell, contracting 256 rows per pass at ~2x
the bf16 rate.  Operands use the DoubleRow 3D AP [K=128, 2, free]
with 16B-aligned pair steps; channels pair (c, c+128) inside chunk
pairs so each pair lives in one partition.  The folded weights ship
pre-scaled by 16 (entries ~N(0, 0.002) would otherwise quantize into
fp8 subnormals); 1/16 is folded into the f32 epilogues.  The -3 shift
keeps exp below 48 (e4m3 max 240; logits are ~N(0,1), max ~6.8) and
cancels exactly in the softmax ratio.  exp runs 1024 wide from a
two-bank PSUM tile to amortize the ACT instruction overhead.
Accumulation is fp32 PSUM everywhere. End-to-end rel l2 vs the f32
reference: ~3.5e-3 (gate 2e-2).
"""

import numpy as np
import ml_dtypes

import concourse.bacc as bacc
import concourse.tile as tile
from concourse import mybir
from concourse import bass_utils

B, C, H, W = 2, 512, 64, 64
HW = H * W              # 4096 spatial positions
P = 128                 # partitions
KC = C // P             # 4 channel chunks; chunk k = (cp, i) = (k//2, k%2)
NCP = 2                 # chunk pairs (DoubleRow contraction = 256 channels)
NCORES = 8
QB = B * HW // NCORES   # 1024 query positions per core
NIH = 2                 # query halves of 512
G = 8                   # groups
GSZ = C // G            # 64 channels / group
EPS = 1e-6
SCALE = float(C) ** -0.5
NJT = HW // P           # 32 key tiles of 128
NTP = NJT // 2          # 16 key tile-pairs (DoubleRow)
SHIFT = 3.0             # exp(logit - SHIFT); cancels in the softmax ratio
WS = 16.0               # host prescale of the folded weights before fp8

F32 = mybir.dt.float32
FP8 = mybir.dt.float8e4
NP8 = ml_dtypes.float8_e4m3
AX = mybir.AxisListType
OP = mybir.AluOpType
AF = mybir.ActivationFunctionType
DR = mybir.MatmulPerfMode.DoubleRow


def _build():
    nc = bacc.Bacc("TRN2", target_bir_lowering=False, debug=False,
                   num_devices=NCORES)

    x8_d = nc.dram_tensor("x8", [P, NCP, 2, HW], FP8, kind="ExternalInput").ap()
    xq_d = nc.dram_tensor("xq", [C, QB], F32, kind="ExternalInput").ap()
    wa_d = nc.dram_tensor("wa8", [P, NCP, 2, C], FP8, kind="ExternalInput").ap()
    wv_d = nc.dram_tensor("wv8", [P, NCP, 2, C], FP8, kind="ExternalInput").ap()
    ones_d = nc.dram_tensor("ones8", [P, 2, P], FP8, kind="ExternalInput").ap()
    vec_d = nc.dram_tensor("vecs", [P, 3 * KC], F32, kind="ExternalInput").ap()
    out_d = nc.dram_tensor("out", [C, QB], F32, kind="ExternalOutput").ap()

    with tile.TileContext(nc) as tc:
        _body(nc, tc, x8_d, xq_d, wa_d, wv_d, ones_d, vec_d, out_d)

    nc.compile()
    return nc


def _body(nc, tc, x8_d, xq_d, wa_d, wv_d, ones_d, vec_d, out_d):
    with (
        tc.tile_pool(name="xbuf", bufs=1) as px,
        tc.tile_pool(name="vt", bufs=1) as pvt,
        tc.tile_pool(name="atb", bufs=1) as pat,
        tc.tile_pool(name="qbuf", bufs=1) as pq,
        tc.tile_pool(name="w8", bufs=2) as pw8,
        tc.tile_pool(name="xq", bufs=1) as pxq,
        tc.tile_pool(name="small", bufs=4) as ps,
    ):
        # ---- persistent tiles ------------------------------------------
        x8t = px.tile([P, NCP, 2, HW], FP8, name="x8t")
        vt8 = pvt.tile([P, NTP, 2, C], FP8, name="vt8")
        at8 = pat.tile([P, NIH, NTP, 2, 512], FP8, name="at8")
        q8t = pq.tile([P, NCP, 2, QB], FP8, name="q8t")
        wa8 = pw8.tile([P, NCP, 2, C], FP8, tag="wa", name="wa8")
        wv8 = pw8.tile([P, NCP, 2, C], FP8, tag="wv", name="wv8")
        xq_b = pxq.tile([P, KC, QB], F32, name="xqb")
        rb_t = ps.tile([P, NIH, 512], F32, tag="rb", name="rb")
        ones_t = ps.tile([P, 2, P], FP8, tag="ones", name="onest")
        vec_b = ps.tile([P, 3 * KC], F32, tag="vec", name="vecb")

        # small/weight DMAs on the gpsimd queue; x8 chunks on sync.  xq is
        # only needed by the final epilogue - it is issued from the vector
        # queue after the q8 epilogues so it doesn't contend for HBM with
        # the startup-critical x8/weight loads.
        nc.gpsimd.dma_start(out=vec_b[:], in_=vec_d[:])
        nc.gpsimd.dma_start(out=wa8[:], in_=wa_d[:])
        nc.gpsimd.dma_start(out=wv8[:], in_=wv_d[:])
        nc.gpsimd.dma_start(out=ones_t[:], in_=ones_d[:])
        NCH = 8
        CHW = HW // NCH
        for ch in range(NCH):
            sl = slice(CHW * ch, CHW * (ch + 1))
            nc.sync.dma_start(out=x8t[:, :, :, sl], in_=x8_d[:, :, :, sl])
        # behind the x8 chunks on the sync ring: starts only once x8 is in
        nc.sync.dma_start(out=xq_b[:],
                          in_=xq_d.rearrange("(k p) n -> p k n", p=P))

        bqe_t = [vec_b[:, m:m + 1] for m in range(KC)]           # 16*(A^T t + Wk^T bq)
        sc16_t = [vec_b[:, 4 + m:5 + m] for m in range(KC)]      # s_cout / 16
        bpe_t = [vec_b[:, 8 + m:9 + m] for m in range(KC)]       # Pv t + Wp bv + bp

        # warm the exp table during the DMAs
        nsh_t = ps.tile([P, 1], F32, tag="nsh", name="nsh")
        nc.gpsimd.memset(nsh_t[:], -float(SHIFT))
        warm = ps.tile([G, 1], F32, tag="warm", name="warm")
        nc.scalar.activation(out=warm[:], in_=nsh_t[0:G, :], func=AF.Exp,
                             scale=SCALE)

        with tc.tile_pool(name="convps", bufs=4, space="PSUM") as pcv:
            # throwaway matmuls as soon as the weights land: ~2us of
            # sustained PE activity flips the HAM clock gate to the full
            # 2.4 GHz rate before the real matmul stream begins
            for r in range(8):
                trash = pcv.tile([P, 256], F32, tag="cv", name=f"trash{r}")
                nc.tensor.matmul(trash[:], lhsT=wa8[:, 0, :, 0:P],
                                 rhs=wa8[:, 0, :, 0:256],
                                 start=True, stop=True, perf_mode=DR)

            # ---- q8 = s/16 * (16 A_s^T xq + 16 A^T t) ------------------
            # (first: its DVE epilogues must lead the vt casts in the DVE
            # queue so the S matmuls aren't gated on the cast backlog)
            for m in range(KC):
                for th in range(NIH):
                    qp = pcv.tile([P, 512], F32, tag="cv", name=f"qp{m}{th}")
                    for cp in range(NCP):
                        nc.tensor.matmul(
                            qp[:],
                            lhsT=wa8[:, cp, :, P * m:P * (m + 1)],
                            rhs=x8t[:, cp, :, 512 * th:512 * (th + 1)],
                            start=(cp == 0), stop=(cp == NCP - 1),
                            perf_mode=DR)
                    nc.vector.tensor_scalar(
                        out=q8t[:, m // 2, m % 2, 512 * th:512 * (th + 1)],
                        in0=qp[:], scalar1=bqe_t[m], scalar2=sc16_t[m],
                        op0=OP.add, op1=OP.mult)
            # ---- VT = x^T (16 Pv_s)^T, cast back to fp8 with /16 -------
            for jt in range(NJT):
                vp = pcv.tile([P, 512], F32, tag="cv", name=f"vp{jt}")
                for cp in range(NCP):
                    nc.tensor.matmul(
                        vp[:],
                        lhsT=x8t[:, cp, :, P * jt:P * (jt + 1)],
                        rhs=wv8[:, cp],
                        start=(cp == 0), stop=(cp == NCP - 1), perf_mode=DR)
                dst = vt8[:, jt // 2, jt % 2, :]
                if jt % 2 == 0:
                    nc.vector.tensor_scalar_mul(dst, vp[:], 1.0 / WS)
                else:
                    nc.scalar.activation(out=dst, in_=vp[:], func=AF.Copy,
                                         scale=1.0 / WS)

        # ---- attention -------------------------------------------------
        with (
            tc.tile_pool(name="sps", bufs=2, space="PSUM") as psps,
            tc.tile_pool(name="csps", bufs=1, space="PSUM") as pcs,
            tc.tile_pool(name="ops", bufs=3, space="PSUM") as pops,
            tc.tile_pool(name="ob", bufs=4) as pob,
        ):
            # S^T (1024-wide two-bank psum) + one wide exp per tile-pair
            # + PE-accumulated row sums
            for ih in range(NIH):
                i_sl = slice(512 * ih, 512 * (ih + 1))
                cs_ps = pcs.tile([P, 512], F32, tag="cs", name=f"cs{ih}")

                def cs_mm(tp):
                    # row-sum matmul, one tile-pair behind the exps so the
                    # PE never waits on ACT
                    nc.tensor.matmul(
                        cs_ps[:], lhsT=ones_t[:], rhs=at8[:, ih, tp],
                        start=(tp == 0), stop=(tp == NTP - 1), perf_mode=DR)

                for tp in range(NTP):
                    sp = psps.tile([P, 2, 512], F32, tag="sp",
                                   name=f"sp{ih}{tp}")
                    for i2 in range(2):
                        jt = 2 * tp + i2
                        for cp in range(NCP):
                            nc.tensor.matmul(
                                sp[:, i2, :],
                                lhsT=x8t[:, cp, :, P * jt:P * (jt + 1)],
                                rhs=q8t[:, cp, :, i_sl],
                                start=(cp == 0), stop=(cp == NCP - 1),
                                perf_mode=DR)
                    nc.scalar.activation(
                        out=at8[:, ih, tp], in_=sp[:],
                        func=AF.Exp, scale=SCALE, bias=nsh_t[:])
                    if tp > 0:
                        cs_mm(tp - 1)
                cs_mm(NTP - 1)
                nc.vector.reciprocal_approx_fast(
                    out=rb_t[:, ih, :], in_=cs_ps[:])

            # AV (m-major; per-m epilogue + output DMA)
            out_v = out_d.rearrange("(k p) n -> p k n", p=P)
            for ih in range(NIH):
                i_sl = slice(512 * ih, 512 * (ih + 1))
                for m in range(KC):
                    o_ps = pops.tile([P, 512], F32, tag="ops",
                                     name=f"ops{ih}{m}")
                    for tp in range(NTP):
                        nc.tensor.matmul(
                            o_ps[:],
                            lhsT=vt8[:, tp, :, P * m:P * (m + 1)],
                            rhs=at8[:, ih, tp],
                            start=(tp == 0), stop=(tp == NTP - 1),
                            perf_mode=DR)
                    ob = pob.tile([P, 512], F32, tag="ob", name=f"ob{ih}{m}")
                    nc.vector.tensor_tensor(
                        out=ob[:], in0=o_ps[:], in1=rb_t[:, ih, :],
                        op=OP.mult)
                    nc.vector.scalar_tensor_tensor(
                        out=ob[:], in0=ob[:], scalar=bpe_t[m],
                        in1=xq_b[:, m, i_sl], op0=OP.add, op1=OP.add)
                    eng = nc.sync if m % 2 == 0 else nc.gpsimd
                    eng.dma_start(out=out_v[:, m, i_sl], in_=ob[:])


_NC_CACHE = {}


def _get_nc():
    if "nc" not in _NC_CACHE:
        _NC_CACHE["nc"] = _build()
    return _NC_CACHE["nc"]


def prepare(inputs):
    x = np.ascontiguousarray(np.asarray(inputs["x"], np.float32))
    norm_w = np.asarray(inputs["norm_w"], np.float64)
    norm_b = np.asarray(inputs["norm_b"], np.float64)
    bs = {w: np.asarray(inputs["b" + w], np.float64) for w in "qkvp"}
    amat = (np.asarray(inputs["wq"], np.float64).T
            @ np.asarray(inputs["wk"], np.float64))
    pvt = (np.asarray(inputs["wp"], np.float64)
           @ np.asarray(inputs["wv"], np.float64)).T
    bqx = np.asarray(inputs["wk"], np.float64).T @ bs["q"]
    bpx = np.asarray(inputs["wp"], np.float64) @ bs["v"] + bs["p"]

    ones8 = np.ones((P, 2, P), NP8)
    # per-batch GroupNorm stats -> folded scaled weights + bias vectors
    per_b = []
    for b in range(B):
        xb = x[b].reshape(C, HW)
        xg = xb.reshape(G, -1).astype(np.float64)
        mean = xg.mean(1)
        var = xg.var(1)
        s = (norm_w / np.sqrt(var + EPS).repeat(GSZ))        # [C]
        t = norm_b - mean.repeat(GSZ) * s                    # [C]
        # pair layout [p, cp, i, cout]: cin = cp*256 + i*128 + p
        wa8 = np.ascontiguousarray(
            (WS * amat * s[:, None]).astype(np.float32).astype(NP8)
            .reshape(NCP, 2, P, C).transpose(2, 0, 1, 3))
        wv8 = np.ascontiguousarray(
            (WS * pvt * s[:, None]).astype(np.float32).astype(NP8)
            .reshape(NCP, 2, P, C).transpose(2, 0, 1, 3))
        bqe = WS * (amat.T @ t + bqx)                        # [C]
        bpe = pvt.T @ t + bpx                                # [C]
        # [P, 12]: columns 0-3 = bqe chunks, 4-7 = s/16, 8-11 = bpe
        vecs = np.ascontiguousarray(np.concatenate(
            [bqe.reshape(KC, P).T, (s / WS).reshape(KC, P).T,
             bpe.reshape(KC, P).T], axis=1).astype(np.float32))
        per_b.append((wa8, wv8, vecs))

    in_maps = []
    for core in range(NCORES):
        b, qb = divmod(core, NCORES // B)
        wa8, wv8, vecs = per_b[b]
        xb = np.ascontiguousarray(x[b].reshape(C, HW))
        xq = np.ascontiguousarray(xb[:, qb * QB:(qb + 1) * QB])
        # keys permuted so this core's query block is first; softmax over
        # the key axis is permutation-invariant, queries/outputs in order
        xb_perm = np.concatenate(
            [xq, xb[:, :qb * QB], xb[:, (qb + 1) * QB:]], axis=1)
        # fp8 pair layout [p, cp, i, pos]: channel c = cp*256 + i*128 + p
        x8 = np.ascontiguousarray(
            xb_perm.astype(NP8).reshape(NCP, 2, P, HW).transpose(2, 0, 1, 3))
        in_maps.append({
            "x8": x8, "xq": xq, "wa8": wa8, "wv8": wv8,
            "ones8": ones8, "vecs": vecs,
        })
    return in_maps


def assemble(results):
    out = np.empty((B, C, HW), np.float32)
    for core in range(NCORES):
        b, qb = divmod(core, NCORES // B)
        out[b][:, qb * QB:(qb + 1) * QB] = results[core]["out"]
    return out.reshape(B, C, H, W)


def run(inputs, **spmd_kwargs):
    in_maps = prepare(inputs)
    nc = _get_nc()
    res = bass_utils.run_bass_kernel_spmd(nc, in_maps, list(range(NCORES)),
                                          **spmd_kwargs)
    return assemble(res.results), res


def kernel(**inputs):
    out, _ = run(inputs)
    return out
